# revision 2
# baseline (speedup 1.0000x reference)
"""Trainium2 Bass kernel for nn_DependencyLinearLayer — PE one-hot matmul design.

Math (collapsed-H reformulation):
  out[b,i,c,j] = T'[dg[b,i,j], c] + s_log[b,i,c] + t_log[b,j,c] + bias[c]
  T'    = dep_emb @ w_d.T                       [48, 12]
  s_log = x @ (w_s @ s_fc_w).T ; t_log = x @ (w_t @ t_fc_w).T
  bias  = w_s@s_fc_b + w_t@t_fc_b + cls_b       (folded into the t-lane)

The lookup+adds run as ONE bf16 matmul per chunk on the Tensor engine with
a c-packed (i-pair) layout. chunk = 32 i-rows x 16 j -> 256 cols n=(a,jl):
  k-rows 0:12    t-lane: lhsT=[I12|I12], rhs=t_log[j,:]+bias (per-buffer copy)
  k-rows 12:60   T-even: lhsT=[T'|0],    rhs=onehot(dg[i_even,j]) (DMA)
  k-rows 60:96   T-odd[0:36]: lhsT=[0|T'], rhs=onehot(dg[i_odd,j])
  k-rows 96:112  s-lane: lhsT=s-pairs (per-ibg copy), rhs=a-indicator (DMA)
  k-rows 112:124 T-odd[36:48]
  k-rows 124:128 zero lane
  psum [24, 256]: rows 0:12 = out[c, i_even, j], rows 12:24 = out[c, i_odd, j]
One-hot rows are host-marshaled (pure index transform). The layout dodges
the engine start-partition rule (starts must be 0/32/64/96): all engine
copies land at 0 or 96; everything else is DMA or matmul. PSUM drains via
ACT/DVE to bf16 staging per i-block, then DMA out in [i,c,j] rows (512B).
DMA triggers are consolidated (few, large) and split across the SP and
GPSIMD queues — per-trigger sequencer cost (~0.7us) was the v1 bottleneck.

Sharding: 8 cores; core n handles batch b = n//2, i-rows [128*(n%2), +128).
"""

import os
import sys

import numpy as np
import ml_dtypes

for _p in ("/opt/trn_rl_repo",):
    if _p not in sys.path:
        sys.path.insert(0, _p)

import concourse.bass as bass  # noqa: E402
import concourse.tile as tile  # noqa: E402
from concourse import bacc, masks, mybir  # noqa: E402
from concourse.tile import ScopedClock, add_dep_helper  # noqa: E402

B, L, IN, H, C, NDEP = 4, 256, 768, 256, 12, 48
NCORES = 8
RPC = 128          # i-rows per core
NSLOT = 8          # lhsT column slots

BF16 = mybir.dt.bfloat16
NP_BF16 = ml_dtypes.bfloat16

# smalls packing offsets (columns; 256-row tensors are 2-folded)
O_WST, O_AUX, O_CLB, O_DB0, O_DB1, O_I12M, O_I12R, O_END = (
    0, 144, 150, 162, 418, 674, 802, 826)

_MAX_TAIL_WAITS = 1


def _patched_drain_and_barrier(self, tick_clock, wait_clock):
    # The walrus build in this image rejects >1 sync-wait on one CTRL
    # instruction; split the kernel-tail drain waits across nops.
    drain_inst = self.nc.sync.drain()
    wait_clock.add_sem_waits(
        drain_inst.ins, ScopedClock({None: tick_clock.global_clock})
    )
    sync_info = drain_inst.ins.sync_info
    if sync_info is not None and len(sync_info.on_wait) > _MAX_TAIL_WAITS:
        waits = list(sync_info.on_wait)
        sync_info.on_wait = waits[:_MAX_TAIL_WAITS]
        rest = waits[_MAX_TAIL_WAITS:]
        while rest:
            chunk, rest = rest[:_MAX_TAIL_WAITS], rest[_MAX_TAIL_WAITS:]
            nop = self.nc.sync.nop(nofuse=True, hint="tail_drain_split").ins
            nop.sync_info = mybir.SyncInfo(on_wait=chunk, on_update=[])
    self.nc.all_engine_barrier()
    assert self.sems is not None
    popped = self.nc._tile_sem_poison_stack.pop()
    assert popped is self._sem_poison
    self.nc.clear_and_free_semaphores(list(self.sems.allocated().values()))
    self.nc.all_engine_barrier()


tile.TileContext._drain_and_barrier = _patched_drain_and_barrier

_PROGRAM = None


def build_program():
    f32 = mybir.dt.float32
    nc = bacc.Bacc("TRN2", target_bir_lowering=False, debug=False)

    xf = nc.declare_dram_parameter("xf", [128, 6 * L], BF16, isOutput=False)
    xsf = nc.declare_dram_parameter("xsf", [128, 6 * RPC], BF16, isOutput=False)
    sfwf = nc.declare_dram_parameter("sfwf", [128, 2 * IN], BF16, isOutput=False)
    tfwf = nc.declare_dram_parameter("tfwf", [128, 2 * IN], BF16, isOutput=False)
    smalls = nc.declare_dram_parameter("smalls", [128, O_END], BF16, isOutput=False)
    idp2 = nc.declare_dram_parameter("idp2", [16, 4096], BF16, isOutput=False)
    # oh per ibg: rows 0:84 -> rb[12:96], 84:100 -> rb[112:128]; cols (jg,jc,a,jl)
    oh = nc.declare_dram_parameter("oh", [4 * 100, 4096], BF16, isOutput=False)
    out_d = nc.declare_dram_parameter("out", [96, 4096], BF16, isOutput=True)

    with tile.TileContext(nc) as tc:
        with (
            tc.tile_pool(name="const", bufs=1) as cp,
            tc.tile_pool(name="psum", bufs=1, space="PSUM") as pp,
            tc.tile_pool(name="stage", bufs=2) as sp,
        ):
            # ---- consolidated loads, critical first ----
            sm_t = cp.tile([128, O_END], BF16, tag="sm_t")
            nc.sync.dma_start(sm_t[:], smalls[:])
            sfw_t = cp.tile([128, 2 * IN], BF16, tag="sfw_t")
            nc.sync.dma_start(sfw_t[:], sfwf[:])
            tfw_t = cp.tile([128, 2 * IN], BF16, tag="tfw_t")
            nc.sync.dma_start(tfw_t[:], tfwf[:])
            xs_t = cp.tile([128, 6 * RPC], BF16, tag="xs_t")
            nc.sync.dma_start(xs_t[:], xsf[:])
            x_t = cp.tile([128, 6 * L], BF16, tag="x_t")
            x_dma = nc.sync.dma_start(x_t[:], xf[:])

            # OH/idp stream gated behind the critical input loads: the Pool
            # queue runs in program order, so a tiny read of x_t makes every
            # later Pool-queue DMA wait for the critical loads.
            gate_t = cp.tile([1, 1], BF16, tag="gate_t")
            nc.gpsimd.tensor_copy(gate_t[:], x_t[0:1, 0:1])
            rb_all = [cp.tile([128, 4096], BF16, name=f"rba{k}", tag=f"rba{k}")
                      for k in range(2)]
            for k in range(2):
                nc.gpsimd.dma_start(rb_all[k][96:112, :], idp2[:, :])

            def wcol(h0, a, b):
                return sm_t[:, O_WST + h0 * 72 + a: O_WST + h0 * 72 + b]

            def acol(h0, a, b):
                return sm_t[:, O_AUX + h0 * 3 + a: O_AUX + h0 * 3 + b]

            clb_t = sm_t[0:1, O_CLB:O_CLB + 12]

            # ---- TT2 master rows via one PSUM matmul group [128, 24] ----
            ttp = pp.tile([128, 24], f32, tag="pro", bufs=2)
            nc.tensor.matmul(ttp[:], sm_t[0:12, O_I12M:O_I12M + 128],
                             sm_t[0:12, O_I12R:O_I12R + 24], start=True, stop=False)
            nc.tensor.matmul(ttp[:], sm_t[:, O_DB0:O_DB0 + 128], wcol(0, 24, 48),
                             start=False, stop=False)
            nc.tensor.matmul(ttp[:], sm_t[:, O_DB0 + 128:O_DB0 + 256], wcol(1, 24, 48),
                             start=False, stop=False)
            nc.tensor.matmul(ttp[:], sm_t[:, O_DB1:O_DB1 + 128], wcol(0, 48, 72),
                             start=False, stop=False)
            nc.tensor.matmul(ttp[:], sm_t[:, O_DB1 + 128:O_DB1 + 256], wcol(1, 48, 72),
                             start=False, stop=True)
            TT2 = cp.tile([128, 24], BF16, tag="TT2")
            nc.vector.tensor_copy(TT2[:], ttp[:])

            # ---- bias vector [12, 1] = ws@sfb + wt@tfb + clsb ----
            bp = pp.tile([C, 1], f32, tag="pro", bufs=2)
            nc.tensor.matmul(bp[:], wcol(0, 0, 12), acol(0, 0, 1), start=True, stop=False)
            nc.tensor.matmul(bp[:], wcol(1, 0, 12), acol(1, 0, 1), start=False, stop=False)
            nc.tensor.matmul(bp[:], wcol(0, 12, 24), acol(0, 1, 2), start=False, stop=False)
            nc.tensor.matmul(bp[:], wcol(1, 12, 24), acol(1, 1, 2), start=False, stop=False)
            nc.tensor.matmul(bp[:], clb_t, acol(0, 2, 3)[0:1, :], start=False, stop=True)
            bias_sb = cp.tile([C, 1], f32, tag="bias_sb")
            nc.scalar.copy(bias_sb[:], bp[:])

            # ---- combined weights W2: w2_t[:, m*24 + (0:12|12:24)] ----
            w2_t = cp.tile([128, 6 * 24], BF16, tag="w2_t")
            for m in range(6):
                pw = pp.tile([128, 24], f32, tag="pro", bufs=2)
                for h0 in range(2):
                    nc.tensor.matmul(
                        pw[:, 0:C],
                        sfw_t[:, h0 * IN + m * 128: h0 * IN + (m + 1) * 128],
                        wcol(h0, 0, 12),
                        start=(h0 == 0), stop=(h0 == 1),
                    )
                for h0 in range(2):
                    nc.tensor.matmul(
                        pw[:, C:2 * C],
                        tfw_t[:, h0 * IN + m * 128: h0 * IN + (m + 1) * 128],
                        wcol(h0, 12, 24),
                        start=(h0 == 0), stop=(h0 == 1),
                    )
                nc.vector.tensor_copy(w2_t[:, m * 24:(m + 1) * 24], pw[:])

            # ---- projections s_logT [12, 128], t_logT [12, 256] ----
            ps = pp.tile([C, RPC], f32, tag="pro", bufs=2)
            pt = pp.tile([C, L], f32, tag="pro", bufs=2)
            for m in range(6):
                nc.tensor.matmul(
                    ps[:], w2_t[:, m * 24: m * 24 + C], xs_t[:, m * RPC:(m + 1) * RPC],
                    start=(m == 0), stop=(m == 5),
                )
            for m in range(6):
                nc.tensor.matmul(
                    pt[:], w2_t[:, m * 24 + C: m * 24 + 2 * C], x_t[:, m * L:(m + 1) * L],
                    start=(m == 0), stop=(m == 5),
                )
            s_logT = cp.tile([C, RPC], BF16, tag="s_logT")
            nc.scalar.copy(s_logT[:], ps[:])
            t_logT = cp.tile([C, L], BF16, tag="t_logT")
            nc.scalar.copy(t_logT[:], pt[:])

            # ---- t-lane values into rb rows 0:12, bias folded; ACT+DVE halves
            for k in range(2):
                for hf in range(2):
                    dst = (rb_all[k][0:12, hf * 2048:(hf + 1) * 2048]
                           .rearrange("p (jg jc a jl) -> p jg jc a jl",
                                      jg=2, jc=4, a=16))
                    src = (t_logT[:, hf * 128:(hf + 1) * 128]
                           .rearrange("p (jg jc jl) -> p jg jc jl", jg=2, jc=4)
                           .unsqueeze(3).broadcast_to([C, 2, 4, 16, 16]))
                    if hf == 0:
                        nc.scalar.activation(
                            dst, src, mybir.ActivationFunctionType.Identity,
                            bias=bias_sb[:])
                    else:
                        nc.vector.tensor_scalar_add(dst, src, bias_sb[:])

            # ---- sPair [128, 64]: rows 32*ibg+a (a<16) hold the i-pair
            # (2*(16*ibg+a), +1) s_log values at cols 0:12 / 32:44 ----
            id64 = cp.tile([64, 64], BF16, tag="id64")
            masks.make_identity(nc, id64[:])
            sPairT = cp.tile([64, 128], BF16, tag="sPairT")
            nc.gpsimd.memset(sPairT[:], 0.0)
            nc.vector.tensor_copy(
                sPairT[0:12, :].rearrange("p (g w) -> p g w", g=4)[:, :, 0:16],
                s_logT[:, :].rearrange("p (g w) -> p g w", g=4)[:, :, 0:31:2],
            )
            nc.vector.tensor_copy(
                sPairT[32:44, :].rearrange("p (g w) -> p g w", g=4)[:, :, 0:16],
                s_logT[:, :].rearrange("p (g w) -> p g w", g=4)[:, :, 1:32:2],
            )
            psp = pp.tile([128, 64], BF16, tag="pro", bufs=2)
            nc.tensor.transpose(psp[:], sPairT[:], id64[:])
            sPair = cp.tile([128, 64], BF16, tag="sPair")
            nc.vector.tensor_copy(sPair[:], psp[:])

            # ---- lhsT master [128, 8*24]; constant rows written once ----
            lt = cp.tile([128, NSLOT * 24], BF16, tag="lt")
            nc.scalar.copy(
                lt[:].rearrange("p (s c) -> p s c", s=NSLOT),
                TT2[:].unsqueeze(1).broadcast_to([128, NSLOT, 24]),
            )

            # ---- main loop: 4 ibg x (4 jg x 4 chunks) ----
            for ibg in range(4):
                rb = rb_all[ibg % 2]
                nc.gpsimd.dma_start(rb[12:96, :], oh[ibg * 100:ibg * 100 + 84, :])
                nc.gpsimd.dma_start(rb[112:128, :],
                                    oh[ibg * 100 + 84:ibg * 100 + 100, :])
                # s-rows for all 8 slots once per ibg
                nc.vector.tensor_copy(
                    lt[96:112, :].rearrange("p (s g c) -> p s g c", s=NSLOT, g=2),
                    sPair[32 * ibg:32 * ibg + 16, :]
                    .rearrange("p (g c) -> p g c", g=2)[:, :, 0:C]
                    .unsqueeze(1).broadcast_to([16, NSLOT, 2, C]),
                )
                stg = sp.tile([24, 4096], BF16, tag="stg")
                for jg in range(4):
                    pb = pp.tile([24, 1024], f32, tag="pb", bufs=3)
                    for u in range(4):
                        slot = (jg * 4 + u) % NSLOT
                        nc.tensor.matmul(
                            pb[:, u * 256:(u + 1) * 256],
                            lt[:, slot * 24:(slot + 1) * 24],
                            rb[:, jg * 1024 + u * 256: jg * 1024 + (u + 1) * 256],
                            start=True, stop=True,
                        )
                    # drain block -> staging (alternate engines)
                    ddst = (
                        stg[:].rearrange("p (a j) -> p a j", a=16)[
                            :, :, jg * 64:(jg + 1) * 64]
                        .rearrange("p a (u jl) -> p u a jl", u=4)
                    )
                    dsrc = pb[:].rearrange("p (u a jl) -> p u a jl", u=4, a=16)
                    if jg % 2:
                        nc.vector.tensor_copy(ddst, dsrc)
                    else:
                        nc.scalar.copy(ddst, dsrc)
                # raw staging dump; host unshards
                nc.sync.dma_start(out_d[24 * ibg:24 * (ibg + 1), :], stg[:])

    nc.compile()
    return nc


def _build_consts(s_fc_w, s_fc_b, t_fc_w, t_fc_b, dep_emb, cls_w, cls_b):
    cw = np.asarray(cls_w, np.float32)
    wsT = cw[:, 0:H].T
    wtT = cw[:, H:2 * H].T
    wdT = cw[:, 2 * H:].T
    z12 = np.zeros((H, 12), np.float32)
    wst = np.concatenate([wsT, wtT, wdT, z12, z12, wdT], axis=1)  # [H, 72]
    aux = np.concatenate([
        np.asarray(s_fc_b, np.float32).reshape(H, 1),
        np.asarray(t_fc_b, np.float32).reshape(H, 1),
        np.ones((H, 1), np.float32),
    ], axis=1)  # [H, 3]
    dt = np.asarray(dep_emb, np.float32)  # [48, H]
    db0 = np.zeros((H, 128), np.float32)
    db1 = np.zeros((H, 128), np.float32)
    db0[:, 12:60] = dt.T
    db1[:, 60:96] = dt.T[:, 0:36]
    db1[:, 112:124] = dt.T[:, 36:48]

    def fold2(a):  # [256, q] -> [128, 2q]
        q = a.shape[1]
        out = np.zeros((128, 2 * q), np.float32)
        out[:, 0:q] = a[0:128]
        out[:, q:2 * q] = a[128:256]
        return out

    smalls = np.zeros((128, O_END), np.float32)
    smalls[:, O_WST:O_WST + 144] = fold2(wst)
    smalls[:, O_AUX:O_AUX + 6] = fold2(aux)
    smalls[0, O_CLB:O_CLB + 12] = np.asarray(cls_b, np.float32)
    smalls[:, O_DB0:O_DB0 + 256] = fold2(db0)
    smalls[:, O_DB1:O_DB1 + 256] = fold2(db1)
    smalls[0:C, O_I12M + np.arange(C)] = 0.0
    i12m = np.zeros((C, 128), np.float32)
    i12m[np.arange(C), np.arange(C)] = 1.0
    smalls[0:C, O_I12M:O_I12M + 128] = i12m
    smalls[0:C, O_I12R:O_I12R + 24] = np.concatenate(
        [np.eye(C, dtype=np.float32)] * 2, axis=1)
    # a-indicator [16, (jg4, jc4, a16, jl16)]
    idp2 = np.zeros((16, 4, 4, 16, 16), np.float32)
    for a in range(16):
        idp2[a, :, :, a, :] = 1.0
    sfwf = np.asarray(s_fc_w, np.float32).reshape(2, 128, IN)
    sfwf = np.concatenate([sfwf[0], sfwf[1]], axis=1)  # [128, 2*IN]
    tfwf = np.asarray(t_fc_w, np.float32).reshape(2, 128, IN)
    tfwf = np.concatenate([tfwf[0], tfwf[1]], axis=1)
    return {
        "sfwf": np.ascontiguousarray(sfwf).astype(NP_BF16),
        "tfwf": np.ascontiguousarray(tfwf).astype(NP_BF16),
        "smalls": smalls.astype(NP_BF16),
        "idp2": idp2.reshape(16, 4096).astype(NP_BF16),
    }


def _marshal_core(n, input_tensor, dg, consts):
    b, half = n // 2, n % 2
    i0 = half * RPC
    xb = input_tensor[b]  # [L, IN] f32
    dgc = np.asarray(dg[b, i0:i0 + RPC])  # [128, 256]
    # one-hot rows per ibg [100, (jg4, jc4, a16, jl16)]
    dgr = dgc.reshape(4, 16, 2, 4, 4, 16)  # [ibg, a, par, jg, jc, jl]
    oh = np.zeros((4, 100, 4, 4, 16, 16), np.float32)  # [ibg, row, jg, jc, a, jl]
    ibg_i, a_i, jg_i, jc_i, jl_i = np.meshgrid(
        np.arange(4), np.arange(16), np.arange(4), np.arange(4), np.arange(16),
        indexing="ij")
    ve = dgr[:, :, 0, :, :, :]
    vo = dgr[:, :, 1, :, :, :]
    oh[ibg_i, ve, jg_i, jc_i, a_i, jl_i] = 1.0                # rows 0:48
    vo_row = np.where(vo < 36, 48 + vo, 84 + (vo - 36))
    oh[ibg_i, vo_row, jg_i, jc_i, a_i, jl_i] = 1.0            # rows 48:96
    oh = oh.reshape(4 * 100, 4096).astype(NP_BF16)
    xT = xb.T  # [IN, L]
    xf = xT.reshape(6, 128, L).transpose(1, 0, 2).reshape(128, 6 * L)
    xsT = xb[i0:i0 + RPC].T
    xsf = xsT.reshape(6, 128, RPC).transpose(1, 0, 2).reshape(128, 6 * RPC)
    m = {
        "xf": np.ascontiguousarray(xf).astype(NP_BF16),
        "xsf": np.ascontiguousarray(xsf).astype(NP_BF16),
        "oh": oh,
    }
    m.update(consts)
    return m


def kernel(input_tensor, dependency_graph, s_fc_w, s_fc_b, t_fc_w, t_fc_b,
           dep_emb, cls_w, cls_b):
    global _PROGRAM
    from concourse.bass_utils import run_bass_kernel_spmd

    input_tensor = np.asarray(input_tensor, dtype=np.float32)
    dg = np.asarray(dependency_graph)

    consts = _build_consts(s_fc_w, s_fc_b, t_fc_w, t_fc_b, dep_emb, cls_w, cls_b)

    if _PROGRAM is None:
        _PROGRAM = build_program()
    nc = _PROGRAM

    in_maps = [_marshal_core(n, input_tensor, dg, consts) for n in range(NCORES)]
    trace = bool(int(os.environ.get("KERNEL_PROFILE", "0")))
    res = run_bass_kernel_spmd(
        nc, in_maps, core_ids=list(range(NCORES)), trace=trace
    )
    if trace and res.exec_time_ns is not None:
        print(f"HW exec time: {res.exec_time_ns} ns")

    out = np.empty((B, L, C, L), dtype=np.float32)
    for n in range(NCORES):
        b, half = n // 2, n % 2
        i0 = half * RPC
        raw = res.results[n]["out"].astype(np.float32)
        raw = raw.reshape(4, 2, C, 16, L)          # [ibg, par, c, a, j]
        raw = raw.transpose(0, 3, 1, 2, 4).reshape(RPC, C, L)
        out[b, i0:i0 + RPC] = raw
    return out


# revision 3
# speedup vs baseline: 1.0307x; 1.0307x over previous
"""Trainium2 Bass kernel for nn_DependencyLinearLayer — PE one-hot matmul design.

Math (collapsed-H reformulation):
  out[b,i,c,j] = T'[dg[b,i,j], c] + s_log[b,i,c] + (t_log[b,j,c] + bias[c])
  T'    = dep_emb @ w_d.T                       [48, 12]
  s_log = x @ (w_s @ s_fc_w).T ; t_log = x @ (w_t @ t_fc_w).T
  bias  = w_s@s_fc_b + w_t@t_fc_b + cls_b       (folded into t_log)

The lookup+adds run as ONE matmul per chunk on the Tensor engine: bf16
lhsT x fp8 rhs (one-hot/indicator rows are 0/1, exact in fp8).
chunk = 32 i-rows x 16 j -> 256 cols n=(a,jl), a = i-pair index:
  k-rows 0:48    T-even: lhsT=[T'|0], rhs=onehot(dg[i_even,j])   (DMA)
  k-rows 48:64   T-odd d0:16: lhsT=[0|T'], rhs=onehot(dg[i_odd,j])
  k-rows 64:80   t-rows: lhsT=t_log+bias values (copy once per slot),
                 rhs=jl-indicator (const, host-baked into the stream)
  k-rows 80:96   T-odd d16:32
  k-rows 96:112  s-rows: lhsT=s-pairs (copy per ibg), rhs=a-indicator
  k-rows 112:128 T-odd d32:48
  psum [24, 256]: rows 0:12 = out[c, i_even, j], rows 12:24 = out[c, i_odd, j]
The rhs stream is ONE fp8 DMA per ibg (128 rows x 4KB, indicator rows
included); lhsT slots are per-jc (NSLOT=16) so t-rows are written once.
All engine copies start at partitions 0/32/64/96 (hardware rule). PSUM
drains via ACT/DVE/GPSIMD round-robin into bf16 staging, DMA'd out raw;
the host does the final unshard. DMA triggers are spread across the SP,
ACT and Pool queues (~0.7us of sequencer time each).

Sharding: 8 cores; core n handles batch b = n//2, i-rows [128*(n%2), +128).
"""

import os
import sys

import numpy as np
import ml_dtypes

for _p in ("/opt/trn_rl_repo",):
    if _p not in sys.path:
        sys.path.insert(0, _p)

import concourse.bass as bass  # noqa: E402
import concourse.tile as tile  # noqa: E402
from concourse import bacc, masks, mybir  # noqa: E402
from concourse.tile import ScopedClock  # noqa: E402

B, L, IN, H, C, NDEP = 4, 256, 768, 256, 12, 48
NCORES = 8
RPC = 128          # i-rows per core
NSLOT = 16         # lhsT column slots (one per jc)

BF16 = mybir.dt.bfloat16
F8 = mybir.dt.float8e4
NP_BF16 = ml_dtypes.bfloat16
NP_F8 = ml_dtypes.float8_e4m3fn

# smalls packing offsets (columns; 256-row tensors are 2-folded)
O_WST, O_AUX, O_CLB, O_DB0, O_DB1, O_END = 0, 144, 150, 162, 418, 674

_MAX_TAIL_WAITS = 1


def _patched_drain_and_barrier(self, tick_clock, wait_clock):
    # The walrus build in this image rejects >1 sync-wait on one CTRL
    # instruction; split the kernel-tail drain waits across nops.
    drain_inst = self.nc.sync.drain()
    wait_clock.add_sem_waits(
        drain_inst.ins, ScopedClock({None: tick_clock.global_clock})
    )
    sync_info = drain_inst.ins.sync_info
    if sync_info is not None and len(sync_info.on_wait) > _MAX_TAIL_WAITS:
        waits = list(sync_info.on_wait)
        sync_info.on_wait = waits[:_MAX_TAIL_WAITS]
        rest = waits[_MAX_TAIL_WAITS:]
        while rest:
            chunk, rest = rest[:_MAX_TAIL_WAITS], rest[_MAX_TAIL_WAITS:]
            nop = self.nc.sync.nop(nofuse=True, hint="tail_drain_split").ins
            nop.sync_info = mybir.SyncInfo(on_wait=chunk, on_update=[])
    self.nc.all_engine_barrier()
    assert self.sems is not None
    popped = self.nc._tile_sem_poison_stack.pop()
    assert popped is self._sem_poison
    self.nc.clear_and_free_semaphores(list(self.sems.allocated().values()))
    self.nc.all_engine_barrier()


tile.TileContext._drain_and_barrier = _patched_drain_and_barrier

_PROGRAM = None


def build_program():
    f32 = mybir.dt.float32
    nc = bacc.Bacc("TRN2", target_bir_lowering=False, debug=False)

    xf = nc.declare_dram_parameter("xf", [128, 6 * L], BF16, isOutput=False)
    xsf = nc.declare_dram_parameter("xsf", [128, 6 * RPC], BF16, isOutput=False)
    sfwf = nc.declare_dram_parameter("sfwf", [128, 2 * IN], BF16, isOutput=False)
    tfwf = nc.declare_dram_parameter("tfwf", [128, 2 * IN], BF16, isOutput=False)
    smalls = nc.declare_dram_parameter("smalls", [128, O_END], BF16, isOutput=False)
    # full rhs stream per ibg: one-hot + indicator rows, fp8
    oh = nc.declare_dram_parameter("oh", [4 * 128, 4096], F8, isOutput=False)
    out_d = nc.declare_dram_parameter("out", [96, 4096], BF16, isOutput=True)

    with tile.TileContext(nc) as tc:
        with (
            tc.tile_pool(name="const", bufs=1) as cp,
            tc.tile_pool(name="psum", bufs=1, space="PSUM") as pp,
            tc.tile_pool(name="stage", bufs=3) as sp,
        ):
            # ---- consolidated loads, critical first ----
            tfw_t = cp.tile([128, 2 * IN], BF16, tag="tfw_t")
            nc.sync.dma_start(tfw_t[:], tfwf[:])
            sfw_t = cp.tile([128, 2 * IN], BF16, tag="sfw_t")
            nc.sync.dma_start(sfw_t[:], sfwf[:])
            xs_t = cp.tile([128, 6 * RPC], BF16, tag="xs_t")
            nc.sync.dma_start(xs_t[:], xsf[:])
            # smalls+x on the scalar ring, in parallel with the sync ring
            sm_t = cp.tile([128, O_END], BF16, tag="sm_t")
            nc.scalar.dma_start(sm_t[:], smalls[:])
            x_t = cp.tile([128, 6 * L], BF16, tag="x_t")
            nc.scalar.dma_start(x_t[:], xf[:])

            id128 = cp.tile([128, 128], BF16, tag="id128")
            masks.make_identity(nc, id128[:])

            # OH stream gated behind the critical loads: the Pool queue runs
            # in program order, so a tiny read of x_t delays later Pool DMAs.
            gate_t = cp.tile([1, 1], BF16, tag="gate_t")
            nc.gpsimd.tensor_copy(gate_t[:], xs_t[0:1, 0:1])
            rb_all = [cp.tile([128, 4096], F8, name=f"rba{k}", tag=f"rba{k}")
                      for k in range(3)]
            for ib in range(3):
                nc.gpsimd.dma_start(rb_all[ib][:], oh[ib * 128:(ib + 1) * 128, :])

            def wcol(h0, a, b):
                return sm_t[:, O_WST + h0 * 72 + a: O_WST + h0 * 72 + b]

            def acol(h0, a, b):
                return sm_t[:, O_AUX + h0 * 3 + a: O_AUX + h0 * 3 + b]

            clb_t = sm_t[0:1, O_CLB:O_CLB + 12]

            # ---- TT2 master rows via one PSUM matmul group [128, 24] ----
            ttp = pp.tile([128, 24], f32, tag="pro", bufs=2)
            nc.tensor.matmul(ttp[:], sm_t[:, O_DB0:O_DB0 + 128], wcol(0, 24, 48),
                             start=True, stop=False)
            nc.tensor.matmul(ttp[:], sm_t[:, O_DB0 + 128:O_DB0 + 256], wcol(1, 24, 48),
                             start=False, stop=False)
            nc.tensor.matmul(ttp[:], sm_t[:, O_DB1:O_DB1 + 128], wcol(0, 48, 72),
                             start=False, stop=False)
            nc.tensor.matmul(ttp[:], sm_t[:, O_DB1 + 128:O_DB1 + 256], wcol(1, 48, 72),
                             start=False, stop=True)
            TT2 = cp.tile([128, 24], BF16, tag="TT2")
            nc.vector.tensor_copy(TT2[:], ttp[:])

            # ---- bias vector [12, 1] = ws@sfb + wt@tfb + clsb ----
            bp = pp.tile([C, 1], f32, tag="pro", bufs=2)
            nc.tensor.matmul(bp[:], wcol(0, 0, 12), acol(0, 0, 1), start=True, stop=False)
            nc.tensor.matmul(bp[:], wcol(1, 0, 12), acol(1, 0, 1), start=False, stop=False)
            nc.tensor.matmul(bp[:], wcol(0, 12, 24), acol(0, 1, 2), start=False, stop=False)
            nc.tensor.matmul(bp[:], wcol(1, 12, 24), acol(1, 1, 2), start=False, stop=False)
            nc.tensor.matmul(bp[:], clb_t, acol(0, 2, 3)[0:1, :], start=False, stop=True)
            bias_sb = cp.tile([C, 1], BF16, tag="bias_sb")
            nc.scalar.copy(bias_sb[:], bp[:])
            pbr = pp.tile([1, C], BF16, tag="pro", bufs=2)
            nc.tensor.transpose(pbr[:], bias_sb[:], id128[0:C, 0:C])
            bias_row = cp.tile([1, C], BF16, tag="bias_row")
            nc.vector.tensor_copy(bias_row[:], pbr[:])

            # ---- combined weights W2 in one psum tile [128, 144] ----
            # t-columns first: the t-chain is the prolog critical path
            pw2 = pp.tile([128, 144], f32, tag="pro", bufs=2)
            w2_t = cp.tile([128, 144], BF16, tag="w2_t")
            for m in range(6):
                for h0 in range(2):
                    nc.tensor.matmul(
                        pw2[:, m * 24 + C:m * 24 + 2 * C],
                        tfw_t[:, h0 * IN + m * 128: h0 * IN + (m + 1) * 128],
                        wcol(h0, 12, 24),
                        start=(h0 == 0), stop=(h0 == 1),
                    )
            nc.vector.tensor_copy(
                w2_t[:].rearrange("p (m v) -> p m v", m=6)[:, :, C:2 * C],
                pw2[:].rearrange("p (m v) -> p m v", m=6)[:, :, C:2 * C],
            )
            for m in range(6):
                for h0 in range(2):
                    nc.tensor.matmul(
                        pw2[:, m * 24:m * 24 + C],
                        sfw_t[:, h0 * IN + m * 128: h0 * IN + (m + 1) * 128],
                        wcol(h0, 0, 12),
                        start=(h0 == 0), stop=(h0 == 1),
                    )
            nc.vector.tensor_copy(
                w2_t[:].rearrange("p (m v) -> p m v", m=6)[:, :, 0:C],
                pw2[:].rearrange("p (m v) -> p m v", m=6)[:, :, 0:C],
            )

            # ---- projections s_logT [12, 128], t_logT [12, 256]+bias ----
            pt = pp.tile([C, L], f32, tag="pro", bufs=2)
            ps = pp.tile([C, RPC], f32, tag="pro", bufs=2)
            nc.tensor.matmul(
                pt[:], bias_row[:],
                acol(0, 2, 3)[0:1, :].broadcast_to([1, L]),
                start=True, stop=False,
            )
            for m in range(6):
                nc.tensor.matmul(
                    pt[:], w2_t[:, m * 24 + C: m * 24 + 2 * C], x_t[:, m * L:(m + 1) * L],
                    start=False, stop=(m == 5),
                )
            for m in range(6):
                nc.tensor.matmul(
                    ps[:], w2_t[:, m * 24: m * 24 + C], xs_t[:, m * RPC:(m + 1) * RPC],
                    start=(m == 0), stop=(m == 5),
                )

            # ---- tPairX [128, 128]: [32*(jc%4)+jl, 32*(jc//4)+c] = t+b ----
            tXT = cp.tile([128, 128], BF16, tag="tXT")
            nc.gpsimd.memset(tXT[:], 0.0)
            for g in range(4):
                nc.scalar.copy(
                    tXT[32 * g:32 * g + C, :]
                    .rearrange("p (jcm w) -> p jcm w", jcm=4)[:, :, 0:16],
                    pt[:, 64 * g:64 * (g + 1)]
                    .rearrange("p (jcm jl) -> p jcm jl", jcm=4),
                )
            ptp = pp.tile([128, 128], BF16, tag="pro", bufs=2)
            nc.tensor.transpose(ptp[:], tXT[:], id128[:])
            tPairX = ptp

            # ---- sPair [128, 64]: rows 32*ibg+a hold i-pair s_log values
            # at cols 0:12 / 32:44 ----
            sPairT = cp.tile([64, 128], BF16, tag="sPairT")
            nc.gpsimd.memset(sPairT[:], 0.0)
            nc.vector.tensor_copy(
                sPairT[0:12, :].rearrange("p (g w) -> p g w", g=4)[:, :, 0:16],
                ps[:, :].rearrange("p (g w) -> p g w", g=4)[:, :, 0:31:2],
            )
            nc.vector.tensor_copy(
                sPairT[32:44, :].rearrange("p (g w) -> p g w", g=4)[:, :, 0:16],
                ps[:, :].rearrange("p (g w) -> p g w", g=4)[:, :, 1:32:2],
            )
            psp = pp.tile([128, 64], BF16, tag="pro", bufs=2)
            nc.tensor.transpose(psp[:], sPairT[:], id128[0:64, 0:64])
            sPair = psp

            # ---- lhsT masters x2 (ibg parity): T rows once, t-rows per slot
            lts = [cp.tile([128, NSLOT * 24], BF16, name=f"lt{v}", tag=f"lt{v}")
                   for v in range(2)]

            def init_lt(v):
                lt = lts[v]
                nc.scalar.copy(
                    lt[:].rearrange("p (s c) -> p s c", s=NSLOT),
                    TT2[:].unsqueeze(1).broadcast_to([128, NSLOT, 24]),
                )
                for m in range(4):
                    tsrc = (tPairX[32 * m:32 * m + 16, :]
                            .rearrange("p (cb w) -> p cb w", cb=4)[:, :, 0:C]
                            .unsqueeze(2).broadcast_to([16, 4, 2, C]))
                    tdst = (lt[64:80, :]
                            .rearrange("p (s q g c) -> p s q g c", s=4, q=4, g=2)
                            [:, :, m, :, :])
                    if m % 2:
                        nc.scalar.copy(tdst, tsrc)
                    else:
                        nc.vector.tensor_copy(tdst, tsrc)

            init_lt(0)

            # ---- main loop: 4 ibg x (4 jg x 4 chunks) ----
            for ibg in range(4):
                rb = rb_all[ibg % 3]
                lt = lts[ibg % 2]
                if ibg == 1:
                    # prefetch the 4th stream into buffer 0 behind ibg0's use
                    nc.gpsimd.dma_start(rb_all[0][:], oh[384:512, :])
                # s-rows for all 16 slots once per ibg
                nc.vector.tensor_copy(
                    lt[96:112, :].rearrange("p (s g c) -> p s g c", s=NSLOT, g=2),
                    sPair[32 * ibg:32 * ibg + 16, :]
                    .rearrange("p (g c) -> p g c", g=2)[:, :, 0:C]
                    .unsqueeze(1).broadcast_to([16, NSLOT, 2, C]),
                )
                stg = sp.tile([24, 4096], BF16, tag="stg")
                for jg in range(4):
                    pb = pp.tile([24, 1024], f32, tag="pb", bufs=3)
                    for u in range(4):
                        jc = jg * 4 + u
                        nc.tensor.matmul(
                            pb[:, u * 256:(u + 1) * 256],
                            lt[:, jc * 24:(jc + 1) * 24],
                            rb[:, jc * 256:(jc + 1) * 256],
                            start=True, stop=True,
                        )
                    # drain block -> staging (3-engine rotation)
                    ddst = (
                        stg[:].rearrange("p (a j) -> p a j", a=16)[
                            :, :, jg * 64:(jg + 1) * 64]
                        .rearrange("p a (u jl) -> p u a jl", u=4)
                    )
                    dsrc = pb[:].rearrange("p (u a jl) -> p u a jl", u=4, a=16)
                    if (ibg * 4 + jg) % 2:
                        nc.scalar.copy(ddst, dsrc)
                    else:
                        nc.vector.tensor_copy(ddst, dsrc)
                # raw staging dump; host unshards
                nc.sync.dma_start(out_d[24 * ibg:24 * (ibg + 1), :], stg[:])
                if ibg == 0:
                    init_lt(1)

    nc.compile()
    return nc


# rhs stream rows per ibg (cols (jg4, jc4, a16, jl16)):
#   0:48 even one-hot; 48+vo+16*(vo//16) odd one-hot (blocks 48:64, 80:96,
#   112:128); 64:80 jl-indicator; 96:112 a-indicator.
def _build_consts(s_fc_w, s_fc_b, t_fc_w, t_fc_b, dep_emb, cls_w, cls_b):
    cw = np.asarray(cls_w, np.float32)
    wsT = cw[:, 0:H].T
    wtT = cw[:, H:2 * H].T
    wdT = cw[:, 2 * H:].T
    z12 = np.zeros((H, 12), np.float32)
    wst = np.concatenate([wsT, wtT, wdT, z12, z12, wdT], axis=1)  # [H, 72]
    aux = np.concatenate([
        np.asarray(s_fc_b, np.float32).reshape(H, 1),
        np.asarray(t_fc_b, np.float32).reshape(H, 1),
        np.ones((H, 1), np.float32),
    ], axis=1)  # [H, 3]
    dt = np.asarray(dep_emb, np.float32)  # [48, H]
    db0 = np.zeros((H, 128), np.float32)
    db1 = np.zeros((H, 128), np.float32)
    db0[:, 0:48] = dt.T
    db1[:, 48:64] = dt.T[:, 0:16]
    db1[:, 80:96] = dt.T[:, 16:32]
    db1[:, 112:128] = dt.T[:, 32:48]

    def fold2(a):  # [256, q] -> [128, 2q]
        q = a.shape[1]
        out = np.zeros((128, 2 * q), np.float32)
        out[:, 0:q] = a[0:128]
        out[:, q:2 * q] = a[128:256]
        return out

    smalls = np.zeros((128, O_END), np.float32)
    smalls[:, O_WST:O_WST + 144] = fold2(wst)
    smalls[:, O_AUX:O_AUX + 6] = fold2(aux)
    smalls[0, O_CLB:O_CLB + 12] = np.asarray(cls_b, np.float32)
    smalls[:, O_DB0:O_DB0 + 256] = fold2(db0)
    smalls[:, O_DB1:O_DB1 + 256] = fold2(db1)

    sfwf = np.asarray(s_fc_w, np.float32).reshape(2, 128, IN)
    sfwf = np.concatenate([sfwf[0], sfwf[1]], axis=1)  # [128, 2*IN]
    tfwf = np.asarray(t_fc_w, np.float32).reshape(2, 128, IN)
    tfwf = np.concatenate([tfwf[0], tfwf[1]], axis=1)
    return {
        "sfwf": np.ascontiguousarray(sfwf).astype(NP_BF16),
        "tfwf": np.ascontiguousarray(tfwf).astype(NP_BF16),
        "smalls": smalls.astype(NP_BF16),
    }


def _marshal_core(n, input_tensor, dg, consts):
    b, half = n // 2, n % 2
    i0 = half * RPC
    xb = input_tensor[b]  # [L, IN] f32
    dgc = np.asarray(dg[b, i0:i0 + RPC])  # [128, 256]
    dgr = dgc.reshape(4, 16, 2, 4, 4, 16)  # [ibg, a, par, jg, jc, jl]
    oh = np.zeros((4, 128, 4, 4, 16, 16), np.float32)  # [ibg, row, jg, jc, a, jl]
    ibg_i, a_i, jg_i, jc_i, jl_i = np.meshgrid(
        np.arange(4), np.arange(16), np.arange(4), np.arange(4), np.arange(16),
        indexing="ij")
    ve = dgr[:, :, 0, :, :, :]
    vo = dgr[:, :, 1, :, :, :]
    oh[ibg_i, ve, jg_i, jc_i, a_i, jl_i] = 1.0
    vo_row = 48 + vo + 16 * (vo // 16)
    oh[ibg_i, vo_row, jg_i, jc_i, a_i, jl_i] = 1.0
    for jl in range(16):
        oh[:, 64 + jl, :, :, :, jl] = 1.0
    for a in range(16):
        oh[:, 96 + a, :, :, a, :] = 1.0
    oh = oh.reshape(4 * 128, 4096).astype(NP_F8)

    xT = xb.T  # [IN, L]
    xf = xT.reshape(6, 128, L).transpose(1, 0, 2).reshape(128, 6 * L)
    xsT = xb[i0:i0 + RPC].T
    xsf = xsT.reshape(6, 128, RPC).transpose(1, 0, 2).reshape(128, 6 * RPC)
    m = {
        "xf": np.ascontiguousarray(xf).astype(NP_BF16),
        "xsf": np.ascontiguousarray(xsf).astype(NP_BF16),
        "oh": oh,
    }
    m.update(consts)
    return m


def kernel(input_tensor, dependency_graph, s_fc_w, s_fc_b, t_fc_w, t_fc_b,
           dep_emb, cls_w, cls_b):
    global _PROGRAM
    from concourse.bass_utils import run_bass_kernel_spmd

    input_tensor = np.asarray(input_tensor, dtype=np.float32)
    dg = np.asarray(dependency_graph)

    consts = _build_consts(s_fc_w, s_fc_b, t_fc_w, t_fc_b, dep_emb, cls_w, cls_b)

    if _PROGRAM is None:
        _PROGRAM = build_program()
    nc = _PROGRAM

    in_maps = [_marshal_core(n, input_tensor, dg, consts) for n in range(NCORES)]
    trace = bool(int(os.environ.get("KERNEL_PROFILE", "0")))
    res = run_bass_kernel_spmd(
        nc, in_maps, core_ids=list(range(NCORES)), trace=trace
    )
    if trace and res.exec_time_ns is not None:
        print(f"HW exec time: {res.exec_time_ns} ns")

    out = np.empty((B, L, C, L), dtype=np.float32)
    for n in range(NCORES):
        b, half = n // 2, n % 2
        i0 = half * RPC
        raw = res.results[n]["out"].astype(np.float32)
        raw = raw.reshape(4, 2, C, 16, L)          # [ibg, par, c, a, j]
        raw = raw.transpose(0, 3, 1, 2, 4).reshape(RPC, C, L)
        out[b, i0:i0 + RPC] = raw
    return out


# revision 6
# speedup vs baseline: 1.0749x; 1.0429x over previous
"""Trainium2 Bass kernel for nn_DependencyLinearLayer — PE one-hot matmul design.

Math (collapsed-H reformulation):
  out[b,i,c,j] = T'[dg[b,i,j], c] + s_log[b,i,c] + (t_log[b,j,c] + bias[c])
  T'    = dep_emb @ w_d.T                       [48, 12]
  s_log = x @ (w_s @ s_fc_w).T ; t_log = x @ (w_t @ t_fc_w).T
  bias  = w_s@s_fc_b + w_t@t_fc_b + cls_b       (folded into t_log)

The lookup+adds run as ONE matmul per chunk on the Tensor engine: bf16
lhsT x fp8 rhs (one-hot/indicator rows are 0/1, exact in fp8).
chunk = 32 i-rows x 16 j -> 256 cols n=(a,jl), a = i-pair index:
  k-rows 0:48    T-even: lhsT=[T'|0], rhs=onehot(dg[i_even,j])   (DMA)
  k-rows 48:64   T-odd d0:16: lhsT=[0|T'], rhs=onehot(dg[i_odd,j])
  k-rows 64:80   t-rows: lhsT=t_log+bias values (copy once per slot),
                 rhs=jl-indicator (const, host-baked into the stream)
  k-rows 80:96   T-odd d16:32
  k-rows 96:112  s-rows: lhsT=s-pairs (copy per ibg), rhs=a-indicator
  k-rows 112:128 T-odd d32:48
  psum [24, 256]: rows 0:12 = out[c, i_even, j], rows 12:24 = out[c, i_odd, j]
The rhs stream is ONE fp8 DMA per ibg (128 rows x 4KB, indicator rows
included); lhsT slots are per-jc (NSLOT=16) so t-rows are written once.
All engine copies start at partitions 0/32/64/96 (hardware rule). PSUM
drains via ACT/DVE/GPSIMD round-robin into bf16 staging, DMA'd out raw;
the host does the final unshard. DMA triggers are spread across the SP,
ACT and Pool queues (~0.7us of sequencer time each).

Sharding: 8 cores; core n handles batch b = n//2, i-rows [128*(n%2), +128).
"""

import os
import sys

import numpy as np
import ml_dtypes

for _p in ("/opt/trn_rl_repo",):
    if _p not in sys.path:
        sys.path.insert(0, _p)

import concourse.bass as bass  # noqa: E402
import concourse.tile as tile  # noqa: E402
from concourse import bacc, masks, mybir  # noqa: E402
from concourse.tile import ScopedClock, add_dep_helper  # noqa: E402

B, L, IN, H, C, NDEP = 4, 256, 768, 256, 12, 48
NCORES = 8
RPC = 128          # i-rows per core
NSLOT = 16         # lhsT column slots (one per jc)

BF16 = mybir.dt.bfloat16
F8 = mybir.dt.float8e4
NP_BF16 = ml_dtypes.bfloat16
NP_F8 = ml_dtypes.float8_e4m3fn

# smalls packing offsets (columns; 256-row tensors are 2-folded)
O_WST, O_AUX, O_CLB, O_DB0, O_DB1, O_END = 0, 144, 150, 162, 418, 674

_MAX_TAIL_WAITS = 1


def _patched_drain_and_barrier(self, tick_clock, wait_clock):
    # The walrus build in this image rejects >1 sync-wait on one CTRL
    # instruction; split the kernel-tail drain waits across nops.
    drain_inst = self.nc.sync.drain()
    wait_clock.add_sem_waits(
        drain_inst.ins, ScopedClock({None: tick_clock.global_clock})
    )
    sync_info = drain_inst.ins.sync_info
    if sync_info is not None and len(sync_info.on_wait) > _MAX_TAIL_WAITS:
        waits = list(sync_info.on_wait)
        sync_info.on_wait = waits[:_MAX_TAIL_WAITS]
        rest = waits[_MAX_TAIL_WAITS:]
        while rest:
            chunk, rest = rest[:_MAX_TAIL_WAITS], rest[_MAX_TAIL_WAITS:]
            nop = self.nc.sync.nop(nofuse=True, hint="tail_drain_split").ins
            nop.sync_info = mybir.SyncInfo(on_wait=chunk, on_update=[])
    self.nc.all_engine_barrier()
    assert self.sems is not None
    popped = self.nc._tile_sem_poison_stack.pop()
    assert popped is self._sem_poison
    self.nc.clear_and_free_semaphores(list(self.sems.allocated().values()))
    self.nc.all_engine_barrier()


tile.TileContext._drain_and_barrier = _patched_drain_and_barrier

_PROGRAM = None


def build_program():
    f32 = mybir.dt.float32
    nc = bacc.Bacc("TRN2", target_bir_lowering=False, debug=False)

    xf = nc.declare_dram_parameter("xf", [128, 6 * L], BF16, isOutput=False)
    xsf = nc.declare_dram_parameter("xsf", [128, 6 * RPC], BF16, isOutput=False)
    sfwf = nc.declare_dram_parameter("sfwf", [128, 2 * IN], BF16, isOutput=False)
    tfwf = nc.declare_dram_parameter("tfwf", [128, 2 * IN], BF16, isOutput=False)
    smalls = nc.declare_dram_parameter("smalls", [128, O_END], BF16, isOutput=False)
    # full rhs stream per ibg: one-hot + indicator rows, fp8
    oh = nc.declare_dram_parameter("oh", [4 * 128, 4096], F8, isOutput=False)
    out_d = nc.declare_dram_parameter("out", [96, 4096], BF16, isOutput=True)

    with tile.TileContext(nc) as tc:
        with (
            tc.tile_pool(name="const", bufs=1) as cp,
            tc.tile_pool(name="psum", bufs=1, space="PSUM") as pp,
            tc.tile_pool(name="stage", bufs=3) as sp,
        ):
            # ---- consolidated loads, critical first ----
            tfw_t = cp.tile([128, 2 * IN], BF16, tag="tfw_t")
            nc.sync.dma_start(tfw_t[:], tfwf[:])
            sfw_t = cp.tile([128, 2 * IN], BF16, tag="sfw_t")
            nc.sync.dma_start(sfw_t[:], sfwf[:])
            xs_t = cp.tile([128, 6 * RPC], BF16, tag="xs_t")
            xs_dma = nc.sync.dma_start(xs_t[:], xsf[:])
            # smalls+x on the scalar ring, in parallel with the sync ring
            sm_t = cp.tile([128, O_END], BF16, tag="sm_t")
            nc.scalar.dma_start(sm_t[:], smalls[:])
            x_t = cp.tile([128, 6 * L], BF16, tag="x_t")
            nc.scalar.dma_start(x_t[:], xf[:])

            id128 = cp.tile([128, 128], BF16, tag="id128")
            masks.make_identity(nc, id128[:])

            # OH stream explicitly sequenced behind the critical loads
            rb_all = [cp.tile([128, 4096], F8, name=f"rba{k}", tag=f"rba{k}")
                      for k in range(4)]
            prev = xs_dma
            for ib in range(4):
                d = nc.gpsimd.dma_start(rb_all[ib][:], oh[ib * 128:(ib + 1) * 128, :])
                add_dep_helper(d.ins, prev.ins, sync=(ib == 0),
                               reason="OH stream after critical loads")
                prev = d

            def wcol(h0, a, b):
                return sm_t[:, O_WST + h0 * 72 + a: O_WST + h0 * 72 + b]

            def acol(h0, a, b):
                return sm_t[:, O_AUX + h0 * 3 + a: O_AUX + h0 * 3 + b]

            clb_t = sm_t[0:1, O_CLB:O_CLB + 12]

            # ---- TT2 master rows via one PSUM matmul group [128, 24] ----
            ttp = pp.tile([128, 24], f32, tag="pro", bufs=2)
            nc.tensor.matmul(ttp[:], sm_t[:, O_DB0:O_DB0 + 128], wcol(0, 24, 48),
                             start=True, stop=False)
            nc.tensor.matmul(ttp[:], sm_t[:, O_DB0 + 128:O_DB0 + 256], wcol(1, 24, 48),
                             start=False, stop=False)
            nc.tensor.matmul(ttp[:], sm_t[:, O_DB1:O_DB1 + 128], wcol(0, 48, 72),
                             start=False, stop=False)
            nc.tensor.matmul(ttp[:], sm_t[:, O_DB1 + 128:O_DB1 + 256], wcol(1, 48, 72),
                             start=False, stop=True)
            TT2 = cp.tile([128, 24], BF16, tag="TT2")
            nc.vector.tensor_copy(TT2[:], ttp[:])

            # ---- bias vector [12, 1] = ws@sfb + wt@tfb + clsb ----
            bp = pp.tile([C, 1], f32, tag="pro", bufs=2)
            nc.tensor.matmul(bp[:], wcol(0, 0, 12), acol(0, 0, 1), start=True, stop=False)
            nc.tensor.matmul(bp[:], wcol(1, 0, 12), acol(1, 0, 1), start=False, stop=False)
            nc.tensor.matmul(bp[:], wcol(0, 12, 24), acol(0, 1, 2), start=False, stop=False)
            nc.tensor.matmul(bp[:], wcol(1, 12, 24), acol(1, 1, 2), start=False, stop=False)
            nc.tensor.matmul(bp[:], clb_t, acol(0, 2, 3)[0:1, :], start=False, stop=True)
            bias_sb = cp.tile([C, 1], BF16, tag="bias_sb")
            nc.scalar.copy(bias_sb[:], bp[:])
            pbr = pp.tile([1, C], BF16, tag="pro", bufs=2)
            nc.tensor.transpose(pbr[:], bias_sb[:], id128[0:C, 0:C])
            bias_row = cp.tile([1, C], BF16, tag="bias_row")
            nc.vector.tensor_copy(bias_row[:], pbr[:])

            # ---- combined weights W2 in one psum tile [128, 144] ----
            # t-columns first: the t-chain is the prolog critical path
            pw2 = pp.tile([128, 144], f32, tag="pro", bufs=2)
            w2_t = cp.tile([128, 144], BF16, tag="w2_t")
            for m in range(6):
                for h0 in range(2):
                    nc.tensor.matmul(
                        pw2[:, m * 24 + C:m * 24 + 2 * C],
                        tfw_t[:, h0 * IN + m * 128: h0 * IN + (m + 1) * 128],
                        wcol(h0, 12, 24),
                        start=(h0 == 0), stop=(h0 == 1),
                    )
            nc.vector.tensor_copy(
                w2_t[:].rearrange("p (m v) -> p m v", m=6)[:, :, C:2 * C],
                pw2[:].rearrange("p (m v) -> p m v", m=6)[:, :, C:2 * C],
            )
            for m in range(6):
                for h0 in range(2):
                    nc.tensor.matmul(
                        pw2[:, m * 24:m * 24 + C],
                        sfw_t[:, h0 * IN + m * 128: h0 * IN + (m + 1) * 128],
                        wcol(h0, 0, 12),
                        start=(h0 == 0), stop=(h0 == 1),
                    )
            nc.vector.tensor_copy(
                w2_t[:].rearrange("p (m v) -> p m v", m=6)[:, :, 0:C],
                pw2[:].rearrange("p (m v) -> p m v", m=6)[:, :, 0:C],
            )

            # ---- projections s_logT [12, 128], t_logT [12, 256]+bias ----
            pt = pp.tile([C, L], f32, tag="pro", bufs=2)
            ps = pp.tile([C, RPC], f32, tag="pro", bufs=2)
            nc.tensor.matmul(
                pt[:], bias_row[:],
                acol(0, 2, 3)[0:1, :].broadcast_to([1, L]),
                start=True, stop=False,
            )
            for m in range(6):
                nc.tensor.matmul(
                    pt[:], w2_t[:, m * 24 + C: m * 24 + 2 * C], x_t[:, m * L:(m + 1) * L],
                    start=False, stop=(m == 5),
                )
            for m in range(6):
                nc.tensor.matmul(
                    ps[:], w2_t[:, m * 24: m * 24 + C], xs_t[:, m * RPC:(m + 1) * RPC],
                    start=(m == 0), stop=(m == 5),
                )

            # ---- tPairX [128, 128]: [32*(jc%4)+jl, 32*(jc//4)+c] = t+b ----
            tXT = cp.tile([128, 128], BF16, tag="tXT")
            nc.gpsimd.memset(tXT[:], 0.0)
            for g in range(4):
                nc.scalar.copy(
                    tXT[32 * g:32 * g + C, :]
                    .rearrange("p (jcm w) -> p jcm w", jcm=4)[:, :, 0:16],
                    pt[:, 64 * g:64 * (g + 1)]
                    .rearrange("p (jcm jl) -> p jcm jl", jcm=4),
                )
            ptp = pp.tile([128, 128], BF16, tag="pro", bufs=2)
            nc.tensor.transpose(ptp[:], tXT[:], id128[:])
            tPairX = ptp

            # ---- sPair [128, 64]: rows 32*ibg+a hold i-pair s_log values
            # at cols 0:12 / 32:44 ----
            sPairT = cp.tile([64, 128], BF16, tag="sPairT")
            nc.gpsimd.memset(sPairT[:], 0.0)
            nc.vector.tensor_copy(
                sPairT[0:12, :].rearrange("p (g w) -> p g w", g=4)[:, :, 0:16],
                ps[:, :].rearrange("p (g w) -> p g w", g=4)[:, :, 0:31:2],
            )
            nc.vector.tensor_copy(
                sPairT[32:44, :].rearrange("p (g w) -> p g w", g=4)[:, :, 0:16],
                ps[:, :].rearrange("p (g w) -> p g w", g=4)[:, :, 1:32:2],
            )
            psp = pp.tile([128, 64], BF16, tag="pro", bufs=2)
            nc.tensor.transpose(psp[:], sPairT[:], id128[0:64, 0:64])
            sPair = psp

            # ---- lhsT masters x2 (ibg parity): T rows once, t-rows per slot
            lts = [cp.tile([128, NSLOT * 24], BF16, name=f"lt{v}", tag=f"lt{v}")
                   for v in range(2)]

            def init_lt(v):
                lt = lts[v]
                nc.scalar.copy(
                    lt[:].rearrange("p (s c) -> p s c", s=NSLOT),
                    TT2[:].unsqueeze(1).broadcast_to([128, NSLOT, 24]),
                )
                for m in range(4):
                    tsrc = (tPairX[32 * m:32 * m + 16, :]
                            .rearrange("p (cb w) -> p cb w", cb=4)[:, :, 0:C]
                            .unsqueeze(2).broadcast_to([16, 4, 2, C]))
                    tdst = (lt[64:80, :]
                            .rearrange("p (s q g c) -> p s q g c", s=4, q=4, g=2)
                            [:, :, m, :, :])
                    if m % 2:
                        nc.scalar.copy(tdst, tsrc)
                    else:
                        nc.vector.tensor_copy(tdst, tsrc)

            init_lt(0)

            # ---- main loop: 4 ibg x (4 jg x 4 chunks) ----
            def s_copy(ibg):
                lt = lts[ibg % 2]
                nc.vector.tensor_copy(
                    lt[96:112, :].rearrange("p (s g c) -> p s g c", s=NSLOT, g=2),
                    sPair[32 * ibg:32 * ibg + 16, :]
                    .rearrange("p (g c) -> p g c", g=2)[:, :, 0:C]
                    .unsqueeze(1).broadcast_to([16, NSLOT, 2, C]),
                )

            s_copy(0)
            for ibg in range(4):
                rb = rb_all[ibg]
                lt = lts[ibg % 2]
                stg = sp.tile([24, 4096], BF16, tag="stg")
                for jg in range(4):
                    pb = pp.tile([24, 1024], f32, tag="pb", bufs=3)
                    for u in range(4):
                        jc = jg * 4 + u
                        nc.tensor.matmul(
                            pb[:, u * 256:(u + 1) * 256],
                            lt[:, jc * 24:(jc + 1) * 24],
                            rb[:, jc * 256:(jc + 1) * 256],
                            start=True, stop=True,
                        )
                    # drain block -> staging (3-engine rotation)
                    ddst = (
                        stg[:].rearrange("p (a j) -> p a j", a=16)[
                            :, :, jg * 64:(jg + 1) * 64]
                        .rearrange("p a (u jl) -> p u a jl", u=4)
                    )
                    dsrc = pb[:].rearrange("p (u a jl) -> p u a jl", u=4, a=16)
                    if ibg == 3:
                        # split drains + per-jg dumps to shorten the tail
                        nc.scalar.copy(ddst[:, 0:2], dsrc[:, 0:2])
                        nc.vector.tensor_copy(ddst[:, 2:4], dsrc[:, 2:4])
                        nc.sync.dma_start(
                            out_d[72:96, :].rearrange(
                                "p (a j) -> p a j", a=16)[
                                :, :, jg * 64:(jg + 1) * 64],
                            stg[:].rearrange("p (a j) -> p a j", a=16)[
                                :, :, jg * 64:(jg + 1) * 64],
                        )
                    elif (ibg * 4 + jg) in (0, 2, 4, 6, 8, 10, 11):
                        nc.scalar.copy(ddst, dsrc)
                    else:
                        nc.vector.tensor_copy(ddst, dsrc)
                # raw staging dump; host unshards
                if ibg < 3:
                    nc.sync.dma_start(out_d[24 * ibg:24 * (ibg + 1), :], stg[:])
                if ibg == 0:
                    init_lt(1)
                    s_copy(1)
                if ibg in (0, 1):
                    s_copy(ibg + 2)

    nc.compile()
    return nc


# rhs stream rows per ibg (cols (jg4, jc4, a16, jl16)):
#   0:48 even one-hot; 48+vo+16*(vo//16) odd one-hot (blocks 48:64, 80:96,
#   112:128); 64:80 jl-indicator; 96:112 a-indicator.
def _build_consts(s_fc_w, s_fc_b, t_fc_w, t_fc_b, dep_emb, cls_w, cls_b):
    cw = np.asarray(cls_w, np.float32)
    wsT = cw[:, 0:H].T
    wtT = cw[:, H:2 * H].T
    wdT = cw[:, 2 * H:].T
    z12 = np.zeros((H, 12), np.float32)
    wst = np.concatenate([wsT, wtT, wdT, z12, z12, wdT], axis=1)  # [H, 72]
    aux = np.concatenate([
        np.asarray(s_fc_b, np.float32).reshape(H, 1),
        np.asarray(t_fc_b, np.float32).reshape(H, 1),
        np.ones((H, 1), np.float32),
    ], axis=1)  # [H, 3]
    dt = np.asarray(dep_emb, np.float32)  # [48, H]
    db0 = np.zeros((H, 128), np.float32)
    db1 = np.zeros((H, 128), np.float32)
    db0[:, 0:48] = dt.T
    db1[:, 48:64] = dt.T[:, 0:16]
    db1[:, 80:96] = dt.T[:, 16:32]
    db1[:, 112:128] = dt.T[:, 32:48]

    def fold2(a):  # [256, q] -> [128, 2q]
        q = a.shape[1]
        out = np.zeros((128, 2 * q), np.float32)
        out[:, 0:q] = a[0:128]
        out[:, q:2 * q] = a[128:256]
        return out

    smalls = np.zeros((128, O_END), np.float32)
    smalls[:, O_WST:O_WST + 144] = fold2(wst)
    smalls[:, O_AUX:O_AUX + 6] = fold2(aux)
    smalls[0, O_CLB:O_CLB + 12] = np.asarray(cls_b, np.float32)
    smalls[:, O_DB0:O_DB0 + 256] = fold2(db0)
    smalls[:, O_DB1:O_DB1 + 256] = fold2(db1)

    sfwf = np.asarray(s_fc_w, np.float32).reshape(2, 128, IN)
    sfwf = np.concatenate([sfwf[0], sfwf[1]], axis=1)  # [128, 2*IN]
    tfwf = np.asarray(t_fc_w, np.float32).reshape(2, 128, IN)
    tfwf = np.concatenate([tfwf[0], tfwf[1]], axis=1)
    return {
        "sfwf": np.ascontiguousarray(sfwf).astype(NP_BF16),
        "tfwf": np.ascontiguousarray(tfwf).astype(NP_BF16),
        "smalls": smalls.astype(NP_BF16),
    }


def _marshal_core(n, input_tensor, dg, consts):
    b, half = n // 2, n % 2
    i0 = half * RPC
    xb = input_tensor[b]  # [L, IN] f32
    dgc = np.asarray(dg[b, i0:i0 + RPC])  # [128, 256]
    dgr = dgc.reshape(4, 16, 2, 4, 4, 16)  # [ibg, a, par, jg, jc, jl]
    oh = np.zeros((4, 128, 4, 4, 16, 16), np.float32)  # [ibg, row, jg, jc, a, jl]
    ibg_i, a_i, jg_i, jc_i, jl_i = np.meshgrid(
        np.arange(4), np.arange(16), np.arange(4), np.arange(4), np.arange(16),
        indexing="ij")
    ve = dgr[:, :, 0, :, :, :]
    vo = dgr[:, :, 1, :, :, :]
    oh[ibg_i, ve, jg_i, jc_i, a_i, jl_i] = 1.0
    vo_row = 48 + vo + 16 * (vo // 16)
    oh[ibg_i, vo_row, jg_i, jc_i, a_i, jl_i] = 1.0
    for jl in range(16):
        oh[:, 64 + jl, :, :, :, jl] = 1.0
    for a in range(16):
        oh[:, 96 + a, :, :, a, :] = 1.0
    oh = oh.reshape(4 * 128, 4096).astype(NP_F8)

    xT = xb.T  # [IN, L]
    xf = xT.reshape(6, 128, L).transpose(1, 0, 2).reshape(128, 6 * L)
    xsT = xb[i0:i0 + RPC].T
    xsf = xsT.reshape(6, 128, RPC).transpose(1, 0, 2).reshape(128, 6 * RPC)
    m = {
        "xf": np.ascontiguousarray(xf).astype(NP_BF16),
        "xsf": np.ascontiguousarray(xsf).astype(NP_BF16),
        "oh": oh,
    }
    m.update(consts)
    return m


def kernel(input_tensor, dependency_graph, s_fc_w, s_fc_b, t_fc_w, t_fc_b,
           dep_emb, cls_w, cls_b):
    global _PROGRAM
    from concourse.bass_utils import run_bass_kernel_spmd

    input_tensor = np.asarray(input_tensor, dtype=np.float32)
    dg = np.asarray(dependency_graph)

    consts = _build_consts(s_fc_w, s_fc_b, t_fc_w, t_fc_b, dep_emb, cls_w, cls_b)

    if _PROGRAM is None:
        _PROGRAM = build_program()
    nc = _PROGRAM

    in_maps = [_marshal_core(n, input_tensor, dg, consts) for n in range(NCORES)]
    trace = bool(int(os.environ.get("KERNEL_PROFILE", "0")))
    res = run_bass_kernel_spmd(
        nc, in_maps, core_ids=list(range(NCORES)), trace=trace
    )
    if trace and res.exec_time_ns is not None:
        print(f"HW exec time: {res.exec_time_ns} ns")

    out = np.empty((B, L, C, L), dtype=np.float32)
    for n in range(NCORES):
        b, half = n // 2, n % 2
        i0 = half * RPC
        raw = res.results[n]["out"].astype(np.float32)
        raw = raw.reshape(4, 2, C, 16, L)          # [ibg, par, c, a, j]
        raw = raw.transpose(0, 3, 1, 2, 4).reshape(RPC, C, L)
        out[b, i0:i0 + RPC] = raw
    return out
